# revision 28
# baseline (speedup 1.0000x reference)
"""Multi-head attention (B=8, N=1024, DIM=768, H=12, D=64) on 8 TRN2 NeuronCores.

Sharding: data-parallel over batch — core b computes batch element b end-to-end
(no collectives). Inside each core everything is computed with the "transposed
scores" formulation so no on-chip transposes are needed:

  xT [C, N]  (host pre-transposed)
  qkT[c_out, n] = w_qkv_chunk.T @ xT          (PE, accumulate over C chunks)
  v  [n, c]     = xT_chunk.T @ w_qkv_v        (natural layout, + ones column)
  scoresT[m, n] = kT.T-slice @ qT             (K=64)
  expT          = exp(SCALE * scoresT)        (ACT, psum->sbuf, bf16 out)
  po[0:64, n]   = [v_h | 1].T @ expT          (accum over m chunks)
  po[64, n]     = colsum (softmax denominator, via the ones column)
  outT          = po[0:64] * (1/colsum)       (recip on DVE, broadcast via DRAM
                                               stride-0 DMA, multiply on DVE)
  out[n, c_out] = outT_chunk.T @ w_proj + bias

Emission is interleaved so the PE work of pair t+1's qkT tiles overlaps the
ACT-bound softmax of pair t, and each colsum group is normalized as soon as
its 4 heads finish.

Dtypes: matmul inputs bf16 by default (1 elem/cycle on PE vs 2 cycles for
f32r); psum accumulation always fp32. Env flags KERN_{QKV,SCORES,PROJ}_BF16
select f32r instead per stage for higher accuracy at ~2x PE cost.
"""

import os
import sys

for _p in ("/opt/trn_rl_repo",):
    if os.path.isdir(_p) and _p not in sys.path:
        sys.path.insert(0, _p)

import numpy as np

import concourse.bass as bass
import concourse.mybir as mybir
import concourse.tile as tile
from concourse import bacc

B, N, DIM, H, D = 8, 1024, 768, 12, 64
SCALE = D ** -0.5
F32 = mybir.dt.float32
F32R = mybir.dt.float32r
BF16 = mybir.dt.bfloat16
KC = DIM // 128          # 6 contraction chunks of 128
NT = N // 128            # 8 tiles of 128 along sequence
PAIRS = H // 2           # head pairs per 128-partition tile
VW = D + 1               # v width incl. the ones column

QKV_BF16 = os.environ.get("KERN_QKV_BF16", "1") == "1"
SCORES_BF16 = os.environ.get("KERN_SCORES_BF16", "1") == "1"
PROJ_BF16 = os.environ.get("KERN_PROJ_BF16", "1") == "1"

QKV_DT = BF16 if QKV_BF16 else F32R
QK_DT = BF16 if SCORES_BF16 else F32R
PROJ_DT = BF16 if PROJ_BF16 else F32R


def build_nc():
    nc = bacc.Bacc(None, target_bir_lowering=False, debug=False)
    xT_d = nc.declare_dram_parameter("xT", [DIM, N], QKV_DT, isOutput=False)
    wqkv_d = nc.declare_dram_parameter("w_qkv", [DIM, 3 * DIM], QKV_DT, isOutput=False)
    wproj_d = nc.declare_dram_parameter("w_proj", [DIM, DIM], PROJ_DT, isOutput=False)
    bias_d = nc.declare_dram_parameter("b_proj", [DIM], F32, isOutput=False)
    cst_d = nc.declare_dram_parameter("cst_ones", [128, N], F32, isOutput=False)
    out_d = nc.declare_dram_parameter("out", [N, DIM], F32, isOutput=True)

    EXP = mybir.ActivationFunctionType.Exp

    with tile.TileContext(nc) as tc:
        with (
            tc.tile_pool(name="sb", bufs=1) as sb,
            tc.tile_pool(name="ps", bufs=2, space="PSUM") as ps,
            tc.tile_pool(name="dram", bufs=1, space="DRAM") as dpool,
        ):
            # ---- persistent sbuf tiles ----
            w_sb = [sb.tile([128, 3 * DIM], QKV_DT, tag=f"w{k}", name=f"w{k}") for k in range(KC)]
            xT_sb = [sb.tile([128, N], QKV_DT, tag=f"x{k}", name=f"x{k}") for k in range(KC)]
            qkT = [sb.tile([128, N], QK_DT, tag=f"qkT{i}", name=f"qkT{i}") for i in range(2 * PAIRS)]
            vaug = [sb.tile([128, H * VW], BF16, tag=f"vaug{i}", name=f"vaug{i}") for i in range(NT)]
            # per-head softmax denominators: head h -> tile h//4, row 32*(h%4)
            # (compute-engine APs must start at a 32-aligned partition)
            colsum_js = [sb.tile([97, N], F32, tag=f"cs{j}", name=f"cs{j}") for j in range(3)]
            cst_sb = sb.tile([128, N], F32, tag="cst", name="cst")
            wproj_sb = [sb.tile([128, DIM], PROJ_DT, tag=f"wp{i}", name=f"wp{i}") for i in range(KC)]
            bias_sb = sb.tile([128, DIM], F32, tag="bias", name="bias")
            outTu = [sb.tile([128, N], PROJ_DT, tag=f"ou{t}", name=f"outTu{t}") for t in range(PAIRS)]
            recip_d = dpool.tile([H, N], F32, name="recip_d")

            # ---- input DMAs ----
            for k in range(KC):
                nc.sync.dma_start(w_sb[k][:], wqkv_d[k * 128:(k + 1) * 128, :])
                nc.sync.dma_start(xT_sb[k][:], xT_d[k * 128:(k + 1) * 128, :])
            for k in range(KC):
                nc.scalar.dma_start(wproj_sb[k][:], wproj_d[k * 128:(k + 1) * 128, :])
            nc.scalar.dma_start(bias_sb[:], bias_d[None, :].to_broadcast((128, DIM)))
            nc.scalar.dma_start(cst_sb[:], cst_d[:, :])
            for j in range(3):
                nc.vector.tensor_copy(colsum_js[j][:], cst_sb[0:97, :])

            def emit_qkT_chain(co, nch):
                pq = ps.tile([128, 512], F32, tag="sc", name="pq")
                for k in range(KC):
                    nc.tensor.matmul(
                        pq[:],
                        w_sb[k][:, co * 128:(co + 1) * 128],
                        xT_sb[k][:, nch * 512:(nch + 1) * 512],
                        start=(k == 0),
                        stop=(k == KC - 1),
                    )
                nc.vector.tensor_copy(qkT[co][:, nch * 512:(nch + 1) * 512], pq[:])

            def emit_qkT_tile(co):
                # qT/kT c_out tile: tile co holds heads (2co, 2co+1) rows
                for nch in range(2):
                    emit_qkT_chain(co, nch)

            def emit_vaug_tile(nt):
                # v in natural [n, c] layout, strided into vaug with a ones col
                pv = ps.tile([128, DIM], F32, tag="sc", name="pv")
                for k in range(KC):
                    nc.tensor.matmul(
                        pv[:, 0:512],
                        xT_sb[k][:, nt * 128:(nt + 1) * 128],
                        w_sb[k][:, 1536:2048],
                        start=(k == 0),
                        stop=(k == KC - 1),
                    )
                    nc.tensor.matmul(
                        pv[:, 512:768],
                        xT_sb[k][:, nt * 128:(nt + 1) * 128],
                        w_sb[k][:, 2048:2304],
                        start=(k == 0),
                        stop=(k == KC - 1),
                    )
                vv = vaug[nt][:].rearrange("p (h c) -> p h c", h=H)
                nc.vector.tensor_copy(
                    vv[:, 0:8, 0:D],
                    pv[:, 0:512].rearrange("p (h c) -> p h c", c=D),
                )
                nc.vector.tensor_copy(
                    vv[:, 8:12, 0:D],
                    pv[:, 512:768].rearrange("p (h c) -> p h c", c=D),
                )
                nc.vector.tensor_copy(vv[:, :, D:VW], cst_sb[:, 0:H, None])

            def emit_head_pair(t, sprinkle_vaug=False, prefetch=()):
                # both heads of pair t together: their score matmuls go to
                # disjoint PE row groups (rows 0-63 / 64-127) and run
                # concurrently; av matmuls lag one mt step behind the exp so
                # the PE rarely waits on ACT
                po = [
                    [ps.tile([VW, 512], F32, tag="acc", bufs=4, name=f"po{half}{nch}") for nch in range(2)]
                    for half in range(2)
                ]
                LAG = 4
                pf_sched = {}
                if prefetch:
                    base = 4 if sprinkle_vaug else 2
                    for i in range(4):
                        pf_sched[base + i] = (prefetch[i // 2], i % 2)
                exs = [None] * NT
                for mt in range(NT):
                    if mt in pf_sched:
                        emit_qkT_chain(*pf_sched[mt])
                    # both heads' scores for one n-chunk go into one psum
                    # tile, written by two matmuls in disjoint PE row groups
                    # (0-63 / 64-127); the critical section keeps them
                    # adjacent so they execute concurrently
                    pair_ex = []
                    for nch in range(2):
                        psc = ps.tile([128, N], F32, tag="sc", name="psc")
                        with tc.tile_critical():
                            for half in range(2):
                                rs0, rs1 = 64 * half, 64 * (half + 1)
                                nc.tensor.matmul(
                                    psc[:, half * 512:(half + 1) * 512],
                                    qkT[PAIRS + t][rs0:rs1, mt * 128:(mt + 1) * 128],
                                    qkT[t][rs0:rs1, nch * 512:(nch + 1) * 512],
                                    start=True,
                                    stop=True,
                                )
                        ex = sb.tile([128, N], BF16, tag="ex", bufs=10, name="ex")
                        nc.scalar.activation(ex[:], psc[:], EXP, scale=SCALE)
                        pair_ex.append(ex)
                    exs[mt] = pair_ex
                    if sprinkle_vaug:
                        emit_vaug_tile(mt)
                    if mt >= LAG:
                        emit_av(t, po, exs[mt - LAG], mt - LAG)
                for mt in range(NT - LAG, NT):
                    emit_av(t, po, exs[mt], mt)
                for half in range(2):
                    h = 2 * t + half
                    rs0, rs1 = 64 * half, 64 * (half + 1)
                    j, p0 = h // 4, 32 * (h % 4)
                    for nch in range(2):
                        ncol = slice(nch * 512, (nch + 1) * 512)
                        nc.vector.tensor_copy(outTu[t][rs0:rs1, ncol], po[half][nch][0:D, :])
                        nc.vector.tensor_copy(colsum_js[j][p0:p0 + 1, ncol], po[half][nch][D:VW, :])

            def emit_av(t, po, pair_ex, mt):
                # pair_ex[nch] holds [head-A | head-B] halves for n-chunk nch
                for half in range(2):
                    h = 2 * t + half
                    for nch in range(2):
                        nc.tensor.matmul(
                            po[half][nch][:],
                            vaug[mt][:, h * VW:(h + 1) * VW],
                            pair_ex[nch][:, half * 512:(half + 1) * 512],
                            start=(mt == 0),
                            stop=(mt == NT - 1),
                        )

            def emit_norm_group(j):
                # after heads 4j..4j+3: 1/colsum, bounce to DRAM, broadcast
                # each head's recip row over 64 partitions, scale outTu
                nc.vector.reciprocal_approx_fast(colsum_js[j][:], colsum_js[j][:])
                for r in range(4):
                    h = 4 * j + r
                    nc.sync.dma_start(recip_d[h:h + 1, :], colsum_js[j][32 * r:32 * r + 1, :])
                for tt in (2 * j, 2 * j + 1):
                    bc = sb.tile([128, N], F32, tag="bc", bufs=2, name="bc")
                    for half in range(2):
                        h = 2 * tt + half
                        nc.sync.dma_start(
                            bc[64 * half:64 * half + 64, :],
                            recip_d[h:h + 1, :].to_broadcast((64, N)),
                        )
                    nc.vector.tensor_mul(outTu[tt][:], outTu[tt][:], bc[:])

            # ---------------- interleaved emission ----------------
            emit_qkT_tile(0)          # q heads 0,1
            emit_qkT_tile(PAIRS)      # k heads 0,1
            for t in range(PAIRS):
                pf = (t + 1, PAIRS + t + 1) if t + 1 < PAIRS else ()
                emit_head_pair(t, sprinkle_vaug=(t == 0), prefetch=pf)
                if t % 2 == 1:
                    emit_norm_group(t // 2)

            # ---------------- output projection ----------------
            for nt in range(NT):
                pp = [
                    ps.tile([128, 512], F32, tag="acc", bufs=4, name="pp0"),
                    ps.tile([128, 256], F32, tag="acc", bufs=4, name="pp1"),
                ]
                for k in range(KC):
                    nc.tensor.matmul(
                        pp[0][:],
                        outTu[k][:, nt * 128:(nt + 1) * 128],
                        wproj_sb[k][:, 0:512],
                        start=(k == 0),
                        stop=(k == KC - 1),
                    )
                    nc.tensor.matmul(
                        pp[1][:],
                        outTu[k][:, nt * 128:(nt + 1) * 128],
                        wproj_sb[k][:, 512:768],
                        start=(k == 0),
                        stop=(k == KC - 1),
                    )
                ot = sb.tile([128, DIM], F32, tag=f"vaug{nt}", name=f"ot{nt}")
                nc.vector.tensor_add(ot[:, 0:512], pp[0][:], bias_sb[:, 0:512])
                nc.vector.tensor_add(ot[:, 512:768], pp[1][:], bias_sb[:, 512:768])
                nc.sync.dma_start(out_d[nt * 128:(nt + 1) * 128, :], ot[:])

    nc.finalize()
    return nc


_NC = None


def _get_nc():
    global _NC
    if _NC is None:
        _NC = build_nc()
    return _NC


def _in_maps(x, w_qkv, w_proj, b_proj):
    import ml_dtypes

    x = np.ascontiguousarray(np.asarray(x, dtype=np.float32))
    w_qkv = np.ascontiguousarray(np.asarray(w_qkv, dtype=np.float32))
    w_proj = np.ascontiguousarray(np.asarray(w_proj, dtype=np.float32))
    b_proj = np.ascontiguousarray(np.asarray(b_proj, dtype=np.float32))
    if QKV_BF16:
        w_qkv = np.ascontiguousarray(w_qkv.astype(ml_dtypes.bfloat16))
    if PROJ_BF16:
        w_proj = np.ascontiguousarray(w_proj.astype(ml_dtypes.bfloat16))
    cst = np.ones((128, N), dtype=np.float32)
    maps = []
    for b in range(B):
        xT = np.ascontiguousarray(x[b].T)
        if QKV_BF16:
            xT = np.ascontiguousarray(xT.astype(ml_dtypes.bfloat16))
        maps.append(
            {
                "xT": xT,
                "w_qkv": w_qkv,
                "w_proj": w_proj,
                "b_proj": b_proj,
                "cst_ones": cst,
            }
        )
    return maps


def kernel(x, w_qkv, w_proj, b_proj):
    from concourse.bass_utils import run_bass_kernel_spmd

    maps = _in_maps(x, w_qkv, w_proj, b_proj)
    res = run_bass_kernel_spmd(_get_nc(), maps, list(range(B)))
    return np.stack([res.results[c]["out"] for c in range(B)], axis=0)


if __name__ == "__main__":
    rng = np.random.default_rng(0)
    x = rng.standard_normal((B, N, DIM), dtype=np.float32)
    w_qkv = rng.standard_normal((DIM, 3 * DIM), dtype=np.float32) * DIM ** -0.5
    w_proj = rng.standard_normal((DIM, DIM), dtype=np.float32) * DIM ** -0.5
    b_proj = rng.standard_normal((DIM,), dtype=np.float32) * 0.01
    out = kernel(x, w_qkv, w_proj, b_proj)
    print(out.shape, out.dtype)


# revision 29
# speedup vs baseline: 1.5691x; 1.5691x over previous
"""Multi-head attention (B=8, N=1024, DIM=768, H=12, D=64) on 8 TRN2 NeuronCores.

Sharding: data-parallel over batch — core b computes batch element b end-to-end
(no collectives). Inside each core everything is computed with the "transposed
scores" formulation so no on-chip transposes are needed:

  xT [C, N]  (host pre-transposed)
  qkT[c_out, n] = w_qkv_chunk.T @ xT          (PE, accumulate over C chunks)
  v  [n, c]     = xT_chunk.T @ w_qkv_v        (natural layout, + ones column)
  scoresT[m, n] = kT.T-slice @ qT             (K=64)
  expT          = exp(SCALE * scoresT)        (ACT, psum->sbuf, bf16 out)
  po[0:64, n]   = [v_h | 1].T @ expT          (accum over m chunks)
  po[64, n]     = colsum (softmax denominator, via the ones column)
  outT          = po[0:64] * (1/colsum)       (recip on DVE, broadcast via DRAM
                                               stride-0 DMA, multiply on DVE)
  out[n, c_out] = outT_chunk.T @ w_proj + bias

Emission is interleaved so the PE work of pair t+1's qkT tiles overlaps the
ACT-bound softmax of pair t, and each colsum group is normalized as soon as
its 4 heads finish.

Dtypes: matmul inputs bf16 by default (1 elem/cycle on PE vs 2 cycles for
f32r); psum accumulation always fp32. Env flags KERN_{QKV,SCORES,PROJ}_BF16
select f32r instead per stage for higher accuracy at ~2x PE cost.
"""

import os
import sys

for _p in ("/opt/trn_rl_repo",):
    if os.path.isdir(_p) and _p not in sys.path:
        sys.path.insert(0, _p)

import numpy as np

import concourse.bass as bass
import concourse.mybir as mybir
import concourse.tile as tile
from concourse import bacc

B, N, DIM, H, D = 8, 1024, 768, 12, 64
SCALE = D ** -0.5
F32 = mybir.dt.float32
F32R = mybir.dt.float32r
BF16 = mybir.dt.bfloat16
KC = DIM // 128          # 6 contraction chunks of 128
NT = N // 128            # 8 tiles of 128 along sequence
PAIRS = H // 2           # head pairs per 128-partition tile
VW = D + 1               # v width incl. the ones column

QKV_BF16 = os.environ.get("KERN_QKV_BF16", "1") == "1"
SCORES_BF16 = os.environ.get("KERN_SCORES_BF16", "1") == "1"
PROJ_BF16 = os.environ.get("KERN_PROJ_BF16", "1") == "1"

QKV_DT = BF16 if QKV_BF16 else F32R
QK_DT = BF16 if SCORES_BF16 else F32R
PROJ_DT = BF16 if PROJ_BF16 else F32R


def build_nc():
    nc = bacc.Bacc(None, target_bir_lowering=False, debug=False)
    xT_d = nc.declare_dram_parameter("xT", [DIM, N], QKV_DT, isOutput=False)
    wqkv_d = nc.declare_dram_parameter("w_qkv", [DIM, 3 * DIM], QKV_DT, isOutput=False)
    wproj_d = nc.declare_dram_parameter("w_proj", [DIM, DIM], PROJ_DT, isOutput=False)
    bias_d = nc.declare_dram_parameter("b_proj", [DIM], F32, isOutput=False)
    cst_d = nc.declare_dram_parameter("cst_ones", [128, N], F32, isOutput=False)
    out_d = nc.declare_dram_parameter("out", [N, DIM], F32, isOutput=True)

    EXP = mybir.ActivationFunctionType.Exp

    with tile.TileContext(nc) as tc:
        with (
            tc.tile_pool(name="sb", bufs=1) as sb,
            tc.tile_pool(name="ps", bufs=2, space="PSUM") as ps,
            tc.tile_pool(name="dram", bufs=1, space="DRAM") as dpool,
        ):
            # ---- persistent sbuf tiles ----
            w_sb = [sb.tile([128, 3 * DIM], QKV_DT, tag=f"w{k}", name=f"w{k}") for k in range(KC)]
            xT_sb = [sb.tile([128, N], QKV_DT, tag=f"x{k}", name=f"x{k}") for k in range(KC)]
            qkT = [sb.tile([128, N], QK_DT, tag=f"qkT{i}", name=f"qkT{i}") for i in range(2 * PAIRS)]
            vaug = [sb.tile([128, H * VW], BF16, tag=f"vaug{i}", name=f"vaug{i}") for i in range(NT)]
            # per-head softmax denominators: head h -> tile h//4, row 32*(h%4)
            # (compute-engine APs must start at a 32-aligned partition)
            colsum_js = [sb.tile([97, N], F32, tag=f"cs{j}", name=f"cs{j}") for j in range(3)]
            cst_sb = sb.tile([128, N], F32, tag="cst", name="cst")
            wproj_sb = [sb.tile([128, DIM], PROJ_DT, tag=f"wp{i}", name=f"wp{i}") for i in range(KC)]
            bias_sb = sb.tile([128, DIM], F32, tag="bias", name="bias")
            outTu = [sb.tile([128, N], PROJ_DT, tag=f"ou{t}", name=f"outTu{t}") for t in range(PAIRS)]
            recip_d = dpool.tile([H, N], F32, name="recip_d")

            # ---- input DMAs ----
            for k in range(KC):
                nc.sync.dma_start(w_sb[k][:], wqkv_d[k * 128:(k + 1) * 128, :])
                nc.sync.dma_start(xT_sb[k][:], xT_d[k * 128:(k + 1) * 128, :])
            for k in range(KC):
                nc.scalar.dma_start(wproj_sb[k][:], wproj_d[k * 128:(k + 1) * 128, :])
            nc.scalar.dma_start(bias_sb[:], bias_d[None, :].to_broadcast((128, DIM)))
            nc.scalar.dma_start(cst_sb[:], cst_d[:, :])
            for j in range(3):
                nc.vector.tensor_copy(colsum_js[j][:], cst_sb[0:97, :])

            def emit_qkT_chain(co, nch):
                pq = ps.tile([128, 512], F32, tag="sc", name="pq")
                for k in range(KC):
                    nc.tensor.matmul(
                        pq[:],
                        w_sb[k][:, co * 128:(co + 1) * 128],
                        xT_sb[k][:, nch * 512:(nch + 1) * 512],
                        start=(k == 0),
                        stop=(k == KC - 1),
                    )
                nc.vector.tensor_copy(qkT[co][:, nch * 512:(nch + 1) * 512], pq[:])

            def emit_qkT_tile(co):
                # qT/kT c_out tile: tile co holds heads (2co, 2co+1) rows
                for nch in range(2):
                    emit_qkT_chain(co, nch)

            def emit_vaug_tile(nt):
                # v in natural [n, c] layout, strided into vaug with a ones col
                pv = ps.tile([128, DIM], F32, tag="sc", name="pv")
                for k in range(KC):
                    nc.tensor.matmul(
                        pv[:, 0:512],
                        xT_sb[k][:, nt * 128:(nt + 1) * 128],
                        w_sb[k][:, 1536:2048],
                        start=(k == 0),
                        stop=(k == KC - 1),
                    )
                    nc.tensor.matmul(
                        pv[:, 512:768],
                        xT_sb[k][:, nt * 128:(nt + 1) * 128],
                        w_sb[k][:, 2048:2304],
                        start=(k == 0),
                        stop=(k == KC - 1),
                    )
                vv = vaug[nt][:].rearrange("p (h c) -> p h c", h=H)
                nc.vector.tensor_copy(
                    vv[:, 0:8, 0:D],
                    pv[:, 0:512].rearrange("p (h c) -> p h c", c=D),
                )
                nc.vector.tensor_copy(
                    vv[:, 8:12, 0:D],
                    pv[:, 512:768].rearrange("p (h c) -> p h c", c=D),
                )
                nc.vector.tensor_copy(vv[:, :, D:VW], cst_sb[:, 0:H, None])

            def emit_head_pair(t, sprinkle_vaug=False, prefetch=()):
                # both heads of pair t together: their score matmuls go to
                # disjoint PE row groups (rows 0-63 / 64-127) and run
                # concurrently; av matmuls lag one mt step behind the exp so
                # the PE rarely waits on ACT
                po = [
                    [ps.tile([VW, 512], F32, tag="acc", bufs=4, name=f"po{half}{nch}") for nch in range(2)]
                    for half in range(2)
                ]
                LAG = 4
                pf_sched = {}
                if prefetch:
                    base = 4 if sprinkle_vaug else 2
                    for i in range(4):
                        pf_sched[base + i] = (prefetch[i // 2], i % 2)
                exs = [None] * NT
                for mt in range(NT):
                    if mt in pf_sched:
                        emit_qkT_chain(*pf_sched[mt])
                    # both heads' scores for one n-chunk go into one psum
                    # tile, written by two matmuls in disjoint PE row groups
                    # (0-63 / 64-127); the critical section keeps them
                    # adjacent so they execute concurrently
                    pair_ex = []
                    for nch in range(2):
                        psc = ps.tile([128, N], F32, tag="sc", name="psc")
                        for half in range(2):
                            rs0, rs1 = 64 * half, 64 * (half + 1)
                            nc.tensor.matmul(
                                psc[:, half * 512:(half + 1) * 512],
                                qkT[PAIRS + t][rs0:rs1, mt * 128:(mt + 1) * 128],
                                qkT[t][rs0:rs1, nch * 512:(nch + 1) * 512],
                                start=True,
                                stop=True,
                            )
                        ex = sb.tile([128, N], BF16, tag="ex", bufs=10, name="ex")
                        nc.scalar.activation(ex[:], psc[:], EXP, scale=SCALE)
                        pair_ex.append(ex)
                    exs[mt] = pair_ex
                    if sprinkle_vaug:
                        emit_vaug_tile(mt)
                    if mt >= LAG:
                        emit_av(t, po, exs[mt - LAG], mt - LAG)
                for mt in range(NT - LAG, NT):
                    emit_av(t, po, exs[mt], mt)
                for half in range(2):
                    h = 2 * t + half
                    rs0, rs1 = 64 * half, 64 * (half + 1)
                    j, p0 = h // 4, 32 * (h % 4)
                    for nch in range(2):
                        ncol = slice(nch * 512, (nch + 1) * 512)
                        nc.vector.tensor_copy(outTu[t][rs0:rs1, ncol], po[half][nch][0:D, :])
                        nc.vector.tensor_copy(colsum_js[j][p0:p0 + 1, ncol], po[half][nch][D:VW, :])

            def emit_av(t, po, pair_ex, mt):
                # pair_ex[nch] holds [head-A | head-B] halves for n-chunk nch
                for half in range(2):
                    h = 2 * t + half
                    for nch in range(2):
                        nc.tensor.matmul(
                            po[half][nch][:],
                            vaug[mt][:, h * VW:(h + 1) * VW],
                            pair_ex[nch][:, half * 512:(half + 1) * 512],
                            start=(mt == 0),
                            stop=(mt == NT - 1),
                        )

            def emit_norm_group(j):
                # after heads 4j..4j+3: 1/colsum, bounce to DRAM, broadcast
                # each head's recip row over 64 partitions, scale outTu
                nc.vector.reciprocal_approx_fast(colsum_js[j][:], colsum_js[j][:])
                for r in range(4):
                    h = 4 * j + r
                    nc.sync.dma_start(recip_d[h:h + 1, :], colsum_js[j][32 * r:32 * r + 1, :])
                for tt in (2 * j, 2 * j + 1):
                    bc = sb.tile([128, N], F32, tag="bc", bufs=2, name="bc")
                    for half in range(2):
                        h = 2 * tt + half
                        nc.sync.dma_start(
                            bc[64 * half:64 * half + 64, :],
                            recip_d[h:h + 1, :].to_broadcast((64, N)),
                        )
                    nc.vector.tensor_mul(outTu[tt][:], outTu[tt][:], bc[:])

            # ---------------- interleaved emission ----------------
            emit_qkT_tile(0)          # q heads 0,1
            emit_qkT_tile(PAIRS)      # k heads 0,1
            for t in range(PAIRS):
                pf = (t + 1, PAIRS + t + 1) if t + 1 < PAIRS else ()
                emit_head_pair(t, sprinkle_vaug=(t == 0), prefetch=pf)
                if t % 2 == 1:
                    emit_norm_group(t // 2)

            # ---------------- output projection ----------------
            for nt in range(NT):
                pp = [
                    ps.tile([128, 512], F32, tag="acc", bufs=4, name="pp0"),
                    ps.tile([128, 256], F32, tag="acc", bufs=4, name="pp1"),
                ]
                for k in range(KC):
                    nc.tensor.matmul(
                        pp[0][:],
                        outTu[k][:, nt * 128:(nt + 1) * 128],
                        wproj_sb[k][:, 0:512],
                        start=(k == 0),
                        stop=(k == KC - 1),
                    )
                    nc.tensor.matmul(
                        pp[1][:],
                        outTu[k][:, nt * 128:(nt + 1) * 128],
                        wproj_sb[k][:, 512:768],
                        start=(k == 0),
                        stop=(k == KC - 1),
                    )
                ot = sb.tile([128, DIM], F32, tag=f"vaug{nt}", name=f"ot{nt}")
                nc.vector.tensor_add(ot[:, 0:512], pp[0][:], bias_sb[:, 0:512])
                nc.vector.tensor_add(ot[:, 512:768], pp[1][:], bias_sb[:, 512:768])
                nc.sync.dma_start(out_d[nt * 128:(nt + 1) * 128, :], ot[:])

    nc.finalize()
    return nc


_NC = None


def _get_nc():
    global _NC
    if _NC is None:
        _NC = build_nc()
    return _NC


def _in_maps(x, w_qkv, w_proj, b_proj):
    import ml_dtypes

    x = np.ascontiguousarray(np.asarray(x, dtype=np.float32))
    w_qkv = np.ascontiguousarray(np.asarray(w_qkv, dtype=np.float32))
    w_proj = np.ascontiguousarray(np.asarray(w_proj, dtype=np.float32))
    b_proj = np.ascontiguousarray(np.asarray(b_proj, dtype=np.float32))
    if QKV_BF16:
        w_qkv = np.ascontiguousarray(w_qkv.astype(ml_dtypes.bfloat16))
    if PROJ_BF16:
        w_proj = np.ascontiguousarray(w_proj.astype(ml_dtypes.bfloat16))
    cst = np.ones((128, N), dtype=np.float32)
    maps = []
    for b in range(B):
        xT = np.ascontiguousarray(x[b].T)
        if QKV_BF16:
            xT = np.ascontiguousarray(xT.astype(ml_dtypes.bfloat16))
        maps.append(
            {
                "xT": xT,
                "w_qkv": w_qkv,
                "w_proj": w_proj,
                "b_proj": b_proj,
                "cst_ones": cst,
            }
        )
    return maps


def kernel(x, w_qkv, w_proj, b_proj):
    from concourse.bass_utils import run_bass_kernel_spmd

    maps = _in_maps(x, w_qkv, w_proj, b_proj)
    res = run_bass_kernel_spmd(_get_nc(), maps, list(range(B)))
    return np.stack([res.results[c]["out"] for c in range(B)], axis=0)


if __name__ == "__main__":
    rng = np.random.default_rng(0)
    x = rng.standard_normal((B, N, DIM), dtype=np.float32)
    w_qkv = rng.standard_normal((DIM, 3 * DIM), dtype=np.float32) * DIM ** -0.5
    w_proj = rng.standard_normal((DIM, DIM), dtype=np.float32) * DIM ** -0.5
    b_proj = rng.standard_normal((DIM,), dtype=np.float32) * 0.01
    out = kernel(x, w_qkv, w_proj, b_proj)
    print(out.shape, out.dtype)


# revision 30
# speedup vs baseline: 1.6197x; 1.0323x over previous
"""Multi-head attention (B=8, N=1024, DIM=768, H=12, D=64) on 8 TRN2 NeuronCores.

Sharding: data-parallel over batch — core b computes batch element b end-to-end
(no collectives). Inside each core everything is computed with the "transposed
scores" formulation so no on-chip transposes are needed:

  xT [C, N]  (host pre-transposed)
  qkT[c_out, n] = w_qkv_chunk.T @ xT          (PE, accumulate over C chunks)
  v  [n, c]     = xT_chunk.T @ w_qkv_v        (natural layout, + ones column)
  scoresT[m, n] = kT.T-slice @ qT             (K=64)
  expT          = exp(SCALE * scoresT)        (ACT, psum->sbuf, bf16 out)
  po[0:64, n]   = [v_h | 1].T @ expT          (accum over m chunks)
  po[64, n]     = colsum (softmax denominator, via the ones column)
  outT          = po[0:64] * (1/colsum)       (recip on DVE, broadcast via DRAM
                                               stride-0 DMA, multiply on DVE)
  out[n, c_out] = outT_chunk.T @ w_proj + bias

Emission is interleaved so the PE work of pair t+1's qkT tiles overlaps the
ACT-bound softmax of pair t, and each colsum group is normalized as soon as
its 4 heads finish.

Dtypes: matmul inputs bf16 by default (1 elem/cycle on PE vs 2 cycles for
f32r); psum accumulation always fp32. Env flags KERN_{QKV,SCORES,PROJ}_BF16
select f32r instead per stage for higher accuracy at ~2x PE cost.
"""

import os
import sys

for _p in ("/opt/trn_rl_repo",):
    if os.path.isdir(_p) and _p not in sys.path:
        sys.path.insert(0, _p)

import numpy as np

import concourse.bass as bass
import concourse.mybir as mybir
import concourse.tile as tile
from concourse import bacc

B, N, DIM, H, D = 8, 1024, 768, 12, 64
SCALE = D ** -0.5
F32 = mybir.dt.float32
F32R = mybir.dt.float32r
BF16 = mybir.dt.bfloat16
KC = DIM // 128          # 6 contraction chunks of 128
NT = N // 128            # 8 tiles of 128 along sequence
PAIRS = H // 2           # head pairs per 128-partition tile
VW = D + 1               # v width incl. the ones column

QKV_BF16 = os.environ.get("KERN_QKV_BF16", "1") == "1"
SCORES_BF16 = os.environ.get("KERN_SCORES_BF16", "1") == "1"
PROJ_BF16 = os.environ.get("KERN_PROJ_BF16", "1") == "1"

QKV_DT = BF16 if QKV_BF16 else F32R
QK_DT = BF16 if SCORES_BF16 else F32R
PROJ_DT = BF16 if PROJ_BF16 else F32R


def build_nc():
    nc = bacc.Bacc(None, target_bir_lowering=False, debug=False)
    xT_d = nc.declare_dram_parameter("xT", [DIM, N], QKV_DT, isOutput=False)
    wqkv_d = nc.declare_dram_parameter("w_qkv", [DIM, 3 * DIM], QKV_DT, isOutput=False)
    wproj_d = nc.declare_dram_parameter("w_proj", [DIM, DIM], PROJ_DT, isOutput=False)
    bias_d = nc.declare_dram_parameter("b_proj", [DIM], F32, isOutput=False)
    cst_d = nc.declare_dram_parameter("cst_ones", [128, N], F32, isOutput=False)
    out_d = nc.declare_dram_parameter("out", [N, DIM], F32, isOutput=True)

    EXP = mybir.ActivationFunctionType.Exp

    with tile.TileContext(nc) as tc:
        with (
            tc.tile_pool(name="sb", bufs=1) as sb,
            tc.tile_pool(name="ps", bufs=2, space="PSUM") as ps,
            tc.tile_pool(name="dram", bufs=1, space="DRAM") as dpool,
        ):
            # ---- persistent sbuf tiles ----
            w_sb = [sb.tile([128, 3 * DIM], QKV_DT, tag=f"w{k}", name=f"w{k}") for k in range(KC)]
            xT_sb = [sb.tile([128, N], QKV_DT, tag=f"x{k}", name=f"x{k}") for k in range(KC)]
            qkT = [sb.tile([128, N], QK_DT, tag=f"qkT{i}", name=f"qkT{i}") for i in range(2 * PAIRS)]
            vaug = [sb.tile([128, H * VW], BF16, tag=f"vaug{i}", name=f"vaug{i}") for i in range(NT)]
            # per-head softmax denominators: head h -> tile h//4, row 32*(h%4)
            # (compute-engine APs must start at a 32-aligned partition)
            colsum_js = [sb.tile([97, N], F32, tag=f"cs{j}", name=f"cs{j}") for j in range(PAIRS)]
            cst_sb = sb.tile([128, N], F32, tag="cst", name="cst")
            wproj_sb = [sb.tile([128, DIM], PROJ_DT, tag=f"wp{i}", name=f"wp{i}") for i in range(KC)]
            bias_sb = sb.tile([128, DIM], F32, tag="bias", name="bias")
            outTu = [sb.tile([128, N], PROJ_DT, tag=f"ou{t}", name=f"outTu{t}") for t in range(PAIRS)]
            recip_d = dpool.tile([H, N], F32, name="recip_d")

            # ---- input DMAs ----
            # w_qkv comes in host-permuted column order [q0,k0,q1,k1,...,v];
            # load pairs-0/1 columns first, then v, then the rest, so the
            # first attention pair can start as early as possible
            for k in range(KC):
                rows = slice(k * 128, (k + 1) * 128)
                nc.sync.dma_start(w_sb[k][:, 0:512], wqkv_d[rows, 0:512])
                nc.sync.dma_start(xT_sb[k][:], xT_d[rows, :])
            for k in range(KC):
                rows = slice(k * 128, (k + 1) * 128)
                nc.sync.dma_start(w_sb[k][:, 1536:2304], wqkv_d[rows, 1536:2304])
            for k in range(KC):
                rows = slice(k * 128, (k + 1) * 128)
                nc.sync.dma_start(w_sb[k][:, 512:1536], wqkv_d[rows, 512:1536])
            for k in range(KC):
                nc.scalar.dma_start(wproj_sb[k][:], wproj_d[k * 128:(k + 1) * 128, :])
            nc.scalar.dma_start(bias_sb[:], bias_d[None, :].to_broadcast((128, DIM)))
            nc.scalar.dma_start(cst_sb[:], cst_d[:, :])
            # warm the ACT exp table while DMAs run
            warm = sb.tile([1, 8], F32, tag="warm", name="warm")
            nc.vector.memset(warm[:], 0.0)
            nc.scalar.activation(warm[:], warm[:], EXP)
            for j in range(PAIRS):
                nc.vector.tensor_copy(colsum_js[j][:], cst_sb[0:97, :])

            def emit_qkT_chain(co, nch):
                # host permutes w_qkv columns to [q0,k0,q1,k1,...,q5,k5,v]
                blk = 2 * co if co < PAIRS else 2 * (co - PAIRS) + 1
                pq = ps.tile([128, 512], F32, tag="sc", name="pq")
                for k in range(KC):
                    nc.tensor.matmul(
                        pq[:],
                        w_sb[k][:, blk * 128:(blk + 1) * 128],
                        xT_sb[k][:, nch * 512:(nch + 1) * 512],
                        start=(k == 0),
                        stop=(k == KC - 1),
                    )
                nc.vector.tensor_copy(qkT[co][:, nch * 512:(nch + 1) * 512], pq[:])

            def emit_qkT_tile(co):
                # qT/kT c_out tile: tile co holds heads (2co, 2co+1) rows
                for nch in range(2):
                    emit_qkT_chain(co, nch)

            def emit_vaug_tile(nt):
                # v in natural [n, c] layout, strided into vaug with a ones col
                pv = ps.tile([128, DIM], F32, tag="sc", name="pv")
                for k in range(KC):
                    nc.tensor.matmul(
                        pv[:, 0:512],
                        xT_sb[k][:, nt * 128:(nt + 1) * 128],
                        w_sb[k][:, 1536:2048],
                        start=(k == 0),
                        stop=(k == KC - 1),
                    )
                    nc.tensor.matmul(
                        pv[:, 512:768],
                        xT_sb[k][:, nt * 128:(nt + 1) * 128],
                        w_sb[k][:, 2048:2304],
                        start=(k == 0),
                        stop=(k == KC - 1),
                    )
                vv = vaug[nt][:].rearrange("p (h c) -> p h c", h=H)
                nc.vector.tensor_copy(
                    vv[:, 0:8, 0:D],
                    pv[:, 0:512].rearrange("p (h c) -> p h c", c=D),
                )
                nc.vector.tensor_copy(
                    vv[:, 8:12, 0:D],
                    pv[:, 512:768].rearrange("p (h c) -> p h c", c=D),
                )
                nc.vector.tensor_copy(vv[:, :, D:VW], cst_sb[:, 0:H, None])

            def emit_head_pair(t, sprinkle_vaug=False, prefetch=()):
                # both heads of pair t together: their score matmuls go to
                # disjoint PE row groups (rows 0-63 / 64-127) and run
                # concurrently; av matmuls lag one mt step behind the exp so
                # the PE rarely waits on ACT
                po = [
                    [ps.tile([VW, 512], F32, tag="acc", bufs=4, name=f"po{half}{nch}") for nch in range(2)]
                    for half in range(2)
                ]
                LAG = 4 if t < PAIRS - 1 else 1
                pf_sched = {}
                if prefetch:
                    base = 4 if sprinkle_vaug else 2
                    for i in range(4):
                        pf_sched[base + i] = (prefetch[i // 2], i % 2)
                exs = [None] * NT
                for mt in range(NT):
                    if mt in pf_sched:
                        emit_qkT_chain(*pf_sched[mt])
                    # both heads' scores for one n-chunk go into one psum
                    # tile, written by two matmuls in disjoint PE row groups
                    # (0-63 / 64-127); the critical section keeps them
                    # adjacent so they execute concurrently
                    pair_ex = []
                    for nch in range(2):
                        psc = ps.tile([128, N], F32, tag="sc", name="psc")
                        for half in range(2):
                            rs0, rs1 = 64 * half, 64 * (half + 1)
                            nc.tensor.matmul(
                                psc[:, half * 512:(half + 1) * 512],
                                qkT[PAIRS + t][rs0:rs1, mt * 128:(mt + 1) * 128],
                                qkT[t][rs0:rs1, nch * 512:(nch + 1) * 512],
                                start=True,
                                stop=True,
                            )
                        ex = sb.tile([128, N], BF16, tag="ex", bufs=10, name="ex")
                        nc.scalar.activation(ex[:], psc[:], EXP, scale=SCALE)
                        pair_ex.append(ex)
                    exs[mt] = pair_ex
                    if sprinkle_vaug:
                        emit_vaug_tile(mt)
                    if mt >= LAG:
                        emit_av(t, po, exs[mt - LAG], mt - LAG)
                for mt in range(NT - LAG, NT):
                    emit_av(t, po, exs[mt], mt)
                for half in range(2):
                    rs0, rs1 = 64 * half, 64 * (half + 1)
                    p0 = 32 * half
                    for nch in range(2):
                        ncol = slice(nch * 512, (nch + 1) * 512)
                        nc.vector.tensor_copy(outTu[t][rs0:rs1, ncol], po[half][nch][0:D, :])
                        nc.vector.tensor_copy(colsum_js[t][p0:p0 + 1, ncol], po[half][nch][D:VW, :])

            def emit_av(t, po, pair_ex, mt):
                # pair_ex[nch] holds [head-A | head-B] halves for n-chunk nch
                for half in range(2):
                    h = 2 * t + half
                    for nch in range(2):
                        nc.tensor.matmul(
                            po[half][nch][:],
                            vaug[mt][:, h * VW:(h + 1) * VW],
                            pair_ex[nch][:, half * 512:(half + 1) * 512],
                            start=(mt == 0),
                            stop=(mt == NT - 1),
                        )

            def emit_norm_pair(t):
                # right after pair t's epilogue: 1/colsum, bounce to DRAM,
                # broadcast each head's recip row over 64 partitions, scale
                nc.vector.reciprocal_approx_fast(colsum_js[t][:], colsum_js[t][:])
                for half in range(2):
                    h = 2 * t + half
                    nc.sync.dma_start(recip_d[h:h + 1, :], colsum_js[t][32 * half:32 * half + 1, :])
                bc = sb.tile([128, N], F32, tag="bc", bufs=2, name="bc")
                for half in range(2):
                    h = 2 * t + half
                    nc.sync.dma_start(
                        bc[64 * half:64 * half + 64, :],
                        recip_d[h:h + 1, :].to_broadcast((64, N)),
                    )
                nc.vector.tensor_mul(outTu[t][:], outTu[t][:], bc[:])

            # ---------------- interleaved emission ----------------
            emit_qkT_tile(0)          # q pair 0
            emit_qkT_tile(PAIRS)      # k pair 0
            emit_qkT_tile(1)          # q pair 1
            emit_qkT_tile(PAIRS + 1)  # k pair 1
            for t in range(PAIRS):
                pf = (t + 2, PAIRS + t + 2) if t + 2 < PAIRS else ()
                emit_head_pair(t, sprinkle_vaug=(t == 0), prefetch=pf)
                emit_norm_pair(t)

            # ---------------- output projection ----------------
            for nt in range(NT):
                pp = [
                    ps.tile([128, 512], F32, tag="acc", bufs=4, name="pp0"),
                    ps.tile([128, 256], F32, tag="acc", bufs=4, name="pp1"),
                ]
                for k in range(KC):
                    nc.tensor.matmul(
                        pp[0][:],
                        outTu[k][:, nt * 128:(nt + 1) * 128],
                        wproj_sb[k][:, 0:512],
                        start=(k == 0),
                        stop=(k == KC - 1),
                    )
                    nc.tensor.matmul(
                        pp[1][:],
                        outTu[k][:, nt * 128:(nt + 1) * 128],
                        wproj_sb[k][:, 512:768],
                        start=(k == 0),
                        stop=(k == KC - 1),
                    )
                ot = sb.tile([128, DIM], F32, tag=f"vaug{nt}", name=f"ot{nt}")
                nc.vector.tensor_add(ot[:, 0:512], pp[0][:], bias_sb[:, 0:512])
                nc.vector.tensor_add(ot[:, 512:768], pp[1][:], bias_sb[:, 512:768])
                nc.sync.dma_start(out_d[nt * 128:(nt + 1) * 128, :], ot[:])

    nc.finalize()
    return nc


_NC = None


def _get_nc():
    global _NC
    if _NC is None:
        _NC = build_nc()
    return _NC


def _in_maps(x, w_qkv, w_proj, b_proj):
    import ml_dtypes

    x = np.ascontiguousarray(np.asarray(x, dtype=np.float32))
    w_qkv = np.ascontiguousarray(np.asarray(w_qkv, dtype=np.float32))
    w_proj = np.ascontiguousarray(np.asarray(w_proj, dtype=np.float32))
    b_proj = np.ascontiguousarray(np.asarray(b_proj, dtype=np.float32))
    # permute w_qkv columns to [q0,k0,q1,k1,...,q5,k5,v] so pair-0/1
    # columns can be DMA'd first
    blocks = []
    for t in range(PAIRS):
        blocks.append(w_qkv[:, t * 128:(t + 1) * 128])
        blocks.append(w_qkv[:, 768 + t * 128:768 + (t + 1) * 128])
    blocks.append(w_qkv[:, 1536:2304])
    w_qkv = np.ascontiguousarray(np.concatenate(blocks, axis=1))
    if QKV_BF16:
        w_qkv = np.ascontiguousarray(w_qkv.astype(ml_dtypes.bfloat16))
    if PROJ_BF16:
        w_proj = np.ascontiguousarray(w_proj.astype(ml_dtypes.bfloat16))
    cst = np.ones((128, N), dtype=np.float32)
    maps = []
    for b in range(B):
        xT = np.ascontiguousarray(x[b].T)
        if QKV_BF16:
            xT = np.ascontiguousarray(xT.astype(ml_dtypes.bfloat16))
        maps.append(
            {
                "xT": xT,
                "w_qkv": w_qkv,
                "w_proj": w_proj,
                "b_proj": b_proj,
                "cst_ones": cst,
            }
        )
    return maps


def kernel(x, w_qkv, w_proj, b_proj):
    from concourse.bass_utils import run_bass_kernel_spmd

    maps = _in_maps(x, w_qkv, w_proj, b_proj)
    res = run_bass_kernel_spmd(_get_nc(), maps, list(range(B)))
    return np.stack([res.results[c]["out"] for c in range(B)], axis=0)


if __name__ == "__main__":
    rng = np.random.default_rng(0)
    x = rng.standard_normal((B, N, DIM), dtype=np.float32)
    w_qkv = rng.standard_normal((DIM, 3 * DIM), dtype=np.float32) * DIM ** -0.5
    w_proj = rng.standard_normal((DIM, DIM), dtype=np.float32) * DIM ** -0.5
    b_proj = rng.standard_normal((DIM,), dtype=np.float32) * 0.01
    out = kernel(x, w_qkv, w_proj, b_proj)
    print(out.shape, out.dtype)
